# revision 3
# baseline (speedup 1.0000x reference)
"""Multi-head attention on 8 TRN2 NeuronCores (SPMD, no collectives).

Problem: nn_MultiHeadAttention — N=4, S=T=2048, E=1024, H=16, D=64.

Sharding: core c handles batch n = c // 2 and head group g = c % 2
(8 heads = 512 features of E).  Each core computes Q/K/V projections for
its head group, attention for its 8 heads, and a partial output
projection against its 512-row slice of Wo; the host sums the two
partial projections per batch and adds bo.  Activations/weights travel
as fp16 (TensorE fp32r rounds to a 10-bit mantissa anyway); PSUM
accumulation is fp32.

Cost-model-driven layout (matmul cost = out free size x cycles/row;
contraction/stationary size is free):
  K^T[f,t], Q^T[f,s]  features-on-partitions projections (as before)
  V[t,f] projection written into v_blk[t, tc, head, 0:64] with an
         all-ones column 64: the AV matmul's ones column then yields
         the softmax denominator as one extra free element (1.5% cost)
  S^T[t,q] = K^T.T Q^T   (K=64 per head; cost-model optimal regardless)
  P^T = exp(S^T/8) fp16  (ScalarE, 1024-wide; the single biggest
         engine load at ~266us — everything else is scheduled around it)
  y[s,d] = sum_t P^T[t,s-chunk].T (V|1)[t,:]   <- FLIPPED AV: stationary
         = exp tile, moving = V|1 [128,65].  M=128 s-rows, K=128 t's per
         step: half the PE cost of the old y^T orientation (55.5us vs
         109us) because no ones-rows padding of M is needed.
  y /= den  (DVE reciprocal + tensor_scalar, denominator is per-
         partition so no partition-broadcast DMA is needed)
  y^T via PE transpose (identity matmul, fp16 PSUM out) + DVE copy
  outT[e,q] = Wo.T y^T  (partial; host sums pairs)
"""

import numpy as np

import concourse.bass as bass
import concourse.tile as tile
from concourse import bacc, mybir
from concourse.bass_utils import run_bass_kernel_spmd

P = 128
E = 1024          # model dim
EL = 512          # features per core (8 heads x 64)
S = 2048          # query length
T = 2048          # kv length
KO = E // P       # 8 contraction chunks for projections
MC = EL // P      # 4 local feature chunks (= head pairs)
TC = T // P       # 16 T chunks
NB = S // 512     # 4 projection column blocks
JB = 4            # attention q blocks
BQ = S // JB      # 512
SC = BQ // P      # 4 s-chunks per q block
HD = 64

F32 = mybir.dt.float32
FP16 = mybir.dt.float16
EXP = mybir.ActivationFunctionType.Exp

NCORES = 8


def _emit(tc_ctx):
    nc = tc_ctx.nc
    tc = tc_ctx

    xqT = nc.dram_tensor("xqT", [E, S], FP16, kind="ExternalInput").ap()
    xkT = nc.dram_tensor("xkT", [E, T], FP16, kind="ExternalInput").ap()
    xvT = nc.dram_tensor("xvT", [E, T], FP16, kind="ExternalInput").ap()
    wq = nc.dram_tensor("wq", [E, EL], FP16, kind="ExternalInput").ap()
    wk = nc.dram_tensor("wk", [E, EL], FP16, kind="ExternalInput").ap()
    wv = nc.dram_tensor("wv", [E, EL], FP16, kind="ExternalInput").ap()
    wo = nc.dram_tensor("wo", [EL, E], FP16, kind="ExternalInput").ap()
    bq = nc.dram_tensor("bq", [EL], F32, kind="ExternalInput").ap()
    bk = nc.dram_tensor("bk", [EL], F32, kind="ExternalInput").ap()
    bv = nc.dram_tensor("bv", [EL], F32, kind="ExternalInput").ap()
    eyed = nc.dram_tensor("eye", [P, P], FP16, kind="ExternalInput").ap()
    outT = nc.dram_tensor("outT", [E, S], F32, kind="ExternalOutput").ap()

    xq3 = xqT.rearrange("(ko p) s -> p ko s", p=P)
    xk3 = xkT.rearrange("(ko p) s -> p ko s", p=P)
    xv3 = xvT.rearrange("(ko p) s -> p ko s", p=P)
    wq3 = wq.rearrange("(ko p) m -> p ko m", p=P)
    wk3 = wk.rearrange("(ko p) m -> p ko m", p=P)
    wv3 = wv.rearrange("(ko p) m -> p ko m", p=P)
    wo3 = wo.rearrange("(c p) e -> p c e", p=P)

    mm = nc.tensor.matmul

    with (
        tc.tile_pool(name="pp", bufs=1) as pp,
        tc.tile_pool(name="ps_pj", bufs=2, space="PSUM") as ps_pj,
        tc.tile_pool(name="ps_sc", bufs=2, space="PSUM") as ps_sc,
        tc.tile_pool(name="ps_av", bufs=2, space="PSUM") as ps_av,
    ):
        # --- constants; warmup exp to preload the ACT table set early ---
        warm_i = pp.tile([P, 16], F32, tag="warm_i")
        nc.vector.memset(warm_i[:], 0.0)
        warm_o = pp.tile([P, 16], FP16, tag="warm_o")
        nc.scalar.activation(warm_o[:], warm_i[:], EXP)
        bq_sb = pp.tile([P, MC], F32, tag="bq_sb")
        nc.sync.dma_start(bq_sb[:], bq.rearrange("(m p) -> p m", p=P))
        bk_sb = pp.tile([P, MC], F32, tag="bk_sb")
        nc.sync.dma_start(bk_sb[:], bk.rearrange("(m p) -> p m", p=P))
        bv_bc = pp.tile([P, EL], F32, tag="bv_bc")
        nc.sync.dma_start(bv_bc[:], bv.unsqueeze(0).to_broadcast((P, EL)))
        eye = pp.tile([P, P], FP16, tag="eye")
        nc.sync.dma_start(eye[:], eyed)

        kt = pp.tile([P, MC, T], FP16, tag="kt")
        qt = pp.tile([P, MC, S], FP16, tag="qt")

        with tc.tile_pool(name="pa", bufs=1) as pa:
            # V with a ones column appended per head: the flipped AV then
            # yields the softmax denominator as free element 64.
            v_blk = pa.tile([P, TC, KO, 65], FP16, tag="v_blk")
            nc.gpsimd.memset(v_blk[:, :, :, 64:65], 1.0)
            # y (normalized, [s-part, global s-chunk, head, d]) and y^T
            y_all = pa.tile([P, S // P, KO, HD], FP16, tag="y_all")

            exp_tiles = {}

            def scores_unit(p, j):
                """Scores + exp for head pair p, q block j (2 heads packed
                on PE row groups; exp covers a tc pair = 1024 wide to
                amortize ScalarE per-instruction overhead)."""
                ea = pa.tile([P, TC, BQ], FP16, tag="exps", bufs=4, name=f"ea{p}_{j}")
                eb = pa.tile([P, TC, BQ], FP16, tag="exps", bufs=4, name=f"eb{p}_{j}")
                qs = slice(j * BQ, (j + 1) * BQ)
                for tcp in range(TC // 2):
                    sa = ps_sc.tile([P, 2, BQ], F32, tag="sc", name=f"sa{p}_{j}_{tcp}")
                    sb = ps_sc.tile([P, 2, BQ], F32, tag="sc", name=f"sb{p}_{j}_{tcp}")
                    for i in range(2):
                        t0 = (2 * tcp + i) * P
                        mm(sa[:, i, :], kt[0:HD, p, t0:t0 + P], qt[0:HD, p, qs])
                        mm(sb[:, i, :], kt[HD:P, p, t0:t0 + P], qt[HD:P, p, qs])
                    nc.scalar.activation(ea[:, 2 * tcp:2 * tcp + 2, :], sa[:],
                                         EXP, scale=0.125)
                    nc.scalar.activation(eb[:, 2 * tcp:2 * tcp + 2, :], sb[:],
                                         EXP, scale=0.125)
                exp_tiles[(p, j)] = (ea, eb)

            def av_unit(p, j):
                """Flipped AV for heads 2p, 2p+1 of q block j: for each
                s-chunk accumulate y[s,0:64]+den over 16 t-chunks, then
                normalize into y_all.  Also transpose this pair's feature
                chunk of y (c == p) as soon as it is normalized."""
                ea, eb = exp_tiles.pop((p, j))
                for hi, et in ((0, ea), (1, eb)):
                    h = 2 * p + hi
                    # [P, SC, 128] fp32 pads the slot to one full bank so
                    # pool packing can't straddle a bank boundary.
                    ya = ps_av.tile([P, SC, P], F32, tag="av",
                                    name=f"ya{p}_{j}_{hi}")
                    rr = pa.tile([P, SC, 1], F32, tag="rr", bufs=2,
                                 name=f"rr{p}_{j}_{hi}")
                    for sc in range(SC):
                        for t in range(TC):
                            mm(ya[:, sc, 0:65],
                               et[:, t, sc * P:(sc + 1) * P],
                               v_blk[:, t, h, :],
                               start=(t == 0), stop=(t == TC - 1))
                        nc.vector.reciprocal(rr[:, sc, :], ya[:, sc, 64:65])
                        nc.vector.tensor_scalar_mul(
                            y_all[:, SC * j + sc, h, :], ya[:, sc, 0:HD],
                            rr[:, sc, 0:1])

            def transpose_pair(p, j, yt):
                """y_all[s, c=p features] -> yt[f, s] via PE transpose."""
                for sc in range(SC):
                    tpf = ps_pj.tile([P, BQ], F32, tag="pj",
                                     name=f"tp{p}_{j}_{sc}")
                    tph = tpf.bitcast(FP16)  # [P, 1024] fp16 view
                    nc.tensor.transpose(tph[:, 0:P],
                                        y_all[:, SC * j + sc, 2 * p:2 * p + 2, :],
                                        eye[:])
                    nc.vector.tensor_copy(yt[:, p, sc * P:(sc + 1) * P],
                                          tph[:, 0:P])

            def outproj_unit(j, yt):
                for m in range(E // P):
                    ot = ps_pj.tile([P, BQ], F32, tag="pj", name=f"ot{m}_{j}")
                    for c in range(MC):
                        mm(ot[:], wo_sb[:, c, m * P:(m + 1) * P], yt[:, c, :],
                           start=(c == 0), stop=(c == MC - 1))
                    os_ = pa.tile([P, BQ], F32, tag="ostage", bufs=2,
                                  name=f"os{m}_{j}")
                    nc.vector.tensor_copy(os_[:], ot[:])
                    nc.sync.dma_start(
                        outT[m * P:(m + 1) * P, j * BQ:(j + 1) * BQ], os_[:])

            units = [(p, j) for j in range(JB) for p in range(MC)]

            # ---------------- projections (scratch pool nested inside pa
            # so the first scores units can interleave) --------------------
            with tc.tile_pool(name="px", bufs=1) as px:
                wk_sb = px.tile([P, KO, EL], FP16, tag="wk_sb")
                nc.sync.dma_start(wk_sb[:, 0:2, :], wk3[:, 0:2, :])
                nc.sync.dma_start(wk_sb[:, 2:KO, :], wk3[:, 2:KO, :])
                wq_sb = px.tile([P, KO, EL], FP16, tag="wq_sb")
                nc.sync.dma_start(wq_sb[:], wq3)

                def proj_qk(x3, w_sb, out_sb, bias_sb, nbs):
                    for nb in nbs:
                        xt = px.tile([P, KO, 512], FP16, tag="xt", bufs=2)
                        for k0, k1 in ((0, 1), (1, 2), (2, 4), (4, 8)):
                            nc.sync.dma_start(
                                xt[:, k0:k1, :],
                                x3[:, k0:k1, 512 * nb:512 * (nb + 1)])
                        for m in range(MC):
                            pt = ps_pj.tile([P, 512], F32, tag="pj")
                            for ko in range(KO):
                                mm(pt[:], w_sb[:, ko, m * P:(m + 1) * P],
                                   xt[:, ko, :], start=(ko == 0),
                                   stop=(ko == KO - 1))
                            nc.vector.tensor_scalar_add(
                                out_sb[:, m, 512 * nb:512 * (nb + 1)], pt[:],
                                bias_sb[:, m:m + 1])

                proj_qk(xk3, wk_sb, kt, bk_sb, range(NB))
                proj_qk(xq3, wq_sb, qt, bq_sb, [0])
                scores_unit(*units[0])
                scores_unit(*units[1])

                # V projection into v_blk (strided per-head dest with the
                # ones column untouched)
                wv_sb = px.tile([P, KO, EL], FP16, tag="wv_sb")
                nc.sync.dma_start(wv_sb[:], wv3)
                for tb in range(4):
                    xv_t = px.tile([P, KO, 512], FP16, tag="xt", bufs=2,
                                   name=f"xv{tb}")
                    nc.sync.dma_start(xv_t[:], xv3[:, :, tb * 512:(tb + 1) * 512])
                    for tt in range(4):
                        t = 4 * tb + tt
                        pt = ps_pj.tile([P, EL], F32, tag="pj", name=f"vp{t}")
                        for ko in range(KO):
                            mm(pt[:], xv_t[:, ko, tt * P:(tt + 1) * P],
                               wv_sb[:, ko, :],
                               start=(ko == 0), stop=(ko == KO - 1))
                        nc.vector.tensor_add(
                            v_blk[:, t, :, 0:HD],
                            pt[:].rearrange("p (h d) -> p h d", h=KO),
                            bv_bc[:].rearrange("p (h d) -> p h d", h=KO))

                wo_sb = pa.tile([P, MC, E], FP16, tag="wo_sb")
                nc.sync.dma_start(wo_sb[:], wo3)

                # main attention loop: scores stay 2 units ahead of AV so
                # ScalarE (exp) always has queued work.
                yt = None
                for idx, (p, j) in enumerate(units):
                    if p == 0:
                        yt = pa.tile([P, MC, BQ], FP16, tag="yt", bufs=2,
                                     name=f"yt{j}")
                        if j + 1 < JB:
                            proj_qk(xq3, wq_sb, qt, bq_sb, [j + 1])
                    av_unit(p, j)
                    transpose_pair(p, j, yt)
                    if p == MC - 1:
                        outproj_unit(j, yt)
                    if idx + 2 < len(units):
                        scores_unit(*units[idx + 2])


_NC_CACHE = None


def _build():
    global _NC_CACHE
    if _NC_CACHE is None:
        nc = bacc.Bacc("TRN2", target_bir_lowering=False, debug=False,
                       enable_asserts=False)
        with tile.TileContext(nc) as t:
            _emit(t)
        nc.compile()
        _NC_CACHE = nc
    return _NC_CACHE


def make_in_maps(query, key, value, Wq, bq, Wk, bk, Wv, bv, Wo):
    def f16(a):
        return np.ascontiguousarray(np.asarray(a, dtype=np.float32)
                                    .astype(np.float16))

    def f32(a):
        return np.ascontiguousarray(np.asarray(a, dtype=np.float32))

    bq, bk, bv = f32(bq), f32(bk), f32(bv)
    query = np.asarray(query, dtype=np.float32)
    key = np.asarray(key, dtype=np.float32)
    value = np.asarray(value, dtype=np.float32)
    Wq, Wk, Wv, Wo = (np.asarray(a, dtype=np.float32) for a in (Wq, Wk, Wv, Wo))
    eye = np.eye(P, dtype=np.float16)

    in_maps = []
    for c in range(NCORES):
        n, g = divmod(c, 2)
        cs = slice(g * EL, (g + 1) * EL)
        in_maps.append({
            "xqT": f16(query[n].T),
            "xkT": f16(key[n].T),
            "xvT": f16(value[n].T),
            "wq": f16(Wq[:, cs]),
            "wk": f16(Wk[:, cs]),
            "wv": f16(Wv[:, cs]),
            "wo": f16(Wo[cs, :]),
            "bq": np.ascontiguousarray(bq[cs]),
            "bk": np.ascontiguousarray(bk[cs]),
            "bv": np.ascontiguousarray(bv[cs]),
            "eye": eye,
        })
    return in_maps


def gather_output(results, bo):
    bo = np.asarray(bo, dtype=np.float32)
    out = np.empty((NCORES // 2, S, E), dtype=np.float32)
    for n in range(NCORES // 2):
        acc = results[2 * n]["outT"] + results[2 * n + 1]["outT"]
        out[n] = acc.T + bo
    return out


def kernel(query, key, value, Wq, bq, Wk, bk, Wv, bv, Wo, bo):
    nc = _build()
    in_maps = make_in_maps(query, key, value, Wq, bq, Wk, bk, Wv, bv, Wo)
    res = run_bass_kernel_spmd(nc, in_maps, core_ids=list(range(NCORES)))
    return gather_output(res.results, bo)


# revision 7
# speedup vs baseline: 1.0254x; 1.0254x over previous
"""Multi-head attention on 8 TRN2 NeuronCores (SPMD, no collectives).

Problem: nn_MultiHeadAttention — N=4, S=T=2048, E=1024, H=16, D=64.

Sharding: core c handles batch n = c // 2 and head group g = c % 2
(8 heads = 512 features of E).  Each core computes Q/K/V projections for
its head group, attention for its 8 heads, and a partial output
projection against its 512-row slice of Wo; the host sums the two
partial projections per batch and adds bo.  Activations/weights travel
as fp16 (TensorE fp32r rounds to a 10-bit mantissa anyway); PSUM
accumulation is fp32.

Cost-model-driven layout (matmul cost = out free size x cycles/row;
contraction/stationary size is free):
  K^T[f,t], Q^T[f,s]  features-on-partitions projections (as before)
  V[t,f] projection written into v_blk[t, tc, head, 0:64] with an
         all-ones column 64: the AV matmul's ones column then yields
         the softmax denominator as one extra free element (1.5% cost)
  S^T[t,q] = K^T.T Q^T   (K=64 per head; cost-model optimal regardless)
  P^T = exp(S^T/8) fp16  (ScalarE, 1024-wide; the single biggest
         engine load at ~266us — everything else is scheduled around it)
  y[s,d] = sum_t P^T[t,s-chunk].T (V|1)[t,:]   <- FLIPPED AV: stationary
         = exp tile, moving = V|1 [128,65].  M=128 s-rows, K=128 t's per
         step: half the PE cost of the old y^T orientation (55.5us vs
         109us) because no ones-rows padding of M is needed.
  y /= den  (DVE reciprocal + tensor_scalar, denominator is per-
         partition so no partition-broadcast DMA is needed)
  y^T via PE transpose (identity matmul, fp16 PSUM out) + DVE copy
  outT[e,q] = Wo.T y^T  (partial; host sums pairs)
"""

import numpy as np

import concourse.bass as bass
import concourse.tile as tile
from concourse import bacc, mybir
from concourse.bass_utils import run_bass_kernel_spmd

P = 128
E = 1024          # model dim
EL = 512          # features per core (8 heads x 64)
S = 2048          # query length
T = 2048          # kv length
KO = E // P       # 8 contraction chunks for projections
MC = EL // P      # 4 local feature chunks (= head pairs)
TC = T // P       # 16 T chunks
NB = S // 512     # 4 projection column blocks
JB = 4            # attention q blocks
BQ = S // JB      # 512
SC = BQ // P      # 4 s-chunks per q block
HD = 64

F32 = mybir.dt.float32
FP16 = mybir.dt.float16
EXP = mybir.ActivationFunctionType.Exp

NCORES = 8


def _emit(tc_ctx):
    nc = tc_ctx.nc
    tc = tc_ctx

    xqT = nc.dram_tensor("xqT", [E, S], FP16, kind="ExternalInput").ap()
    xkT = nc.dram_tensor("xkT", [E, T], FP16, kind="ExternalInput").ap()
    xvT = nc.dram_tensor("xvT", [E, T], FP16, kind="ExternalInput").ap()
    wq = nc.dram_tensor("wq", [E, EL], FP16, kind="ExternalInput").ap()
    wk = nc.dram_tensor("wk", [E, EL], FP16, kind="ExternalInput").ap()
    wv = nc.dram_tensor("wv", [E, EL], FP16, kind="ExternalInput").ap()
    wo = nc.dram_tensor("wo", [EL, E], FP16, kind="ExternalInput").ap()
    bq = nc.dram_tensor("bq", [EL], F32, kind="ExternalInput").ap()
    bk = nc.dram_tensor("bk", [EL], F32, kind="ExternalInput").ap()
    bv = nc.dram_tensor("bv", [EL], F32, kind="ExternalInput").ap()
    eyed = nc.dram_tensor("eye", [P, P], FP16, kind="ExternalInput").ap()
    outT = nc.dram_tensor("outT", [E, S], F32, kind="ExternalOutput").ap()

    xq3 = xqT.rearrange("(ko p) s -> p ko s", p=P)
    xk3 = xkT.rearrange("(ko p) s -> p ko s", p=P)
    xv3 = xvT.rearrange("(ko p) s -> p ko s", p=P)
    wq3 = wq.rearrange("(ko p) m -> p ko m", p=P)
    wk3 = wk.rearrange("(ko p) m -> p ko m", p=P)
    wv3 = wv.rearrange("(ko p) m -> p ko m", p=P)
    wo3 = wo.rearrange("(c p) e -> p c e", p=P)

    mm = nc.tensor.matmul

    with (
        tc.tile_pool(name="pp", bufs=1) as pp,
        tc.tile_pool(name="ps_pj", bufs=2, space="PSUM") as ps_pj,
        tc.tile_pool(name="ps_sc", bufs=2, space="PSUM") as ps_sc,
        tc.tile_pool(name="ps_av", bufs=2, space="PSUM") as ps_av,
    ):
        # --- constants; warmup exp to preload the ACT table set early ---
        warm_i = pp.tile([P, 16], F32, tag="warm_i")
        nc.gpsimd.memset(warm_i[:], 0.0)
        warm_o = pp.tile([P, 16], FP16, tag="warm_o")
        nc.scalar.activation(warm_o[:], warm_i[:], EXP)
        bq_sb = pp.tile([P, MC], F32, tag="bq_sb")
        bk_sb = pp.tile([P, MC], F32, tag="bk_sb")
        bv_bc = pp.tile([P, EL], F32, tag="bv_bc")
        eye = pp.tile([P, P], FP16, tag="eye")

        kt = pp.tile([P, MC, T], FP16, tag="kt")
        qt = pp.tile([P, MC, S], FP16, tag="qt")

        with tc.tile_pool(name="pa", bufs=1) as pa:
            # V with a ones column appended per head: the flipped AV then
            # yields the softmax denominator as free element 64.
            v_blk = pa.tile([P, TC, KO, 65], FP16, tag="v_blk")
            nc.gpsimd.memset(v_blk[:, :, :, 64:65], 1.0)
            # y (normalized, [s-part, global s-chunk, head, d]) and y^T
            y_all = pa.tile([P, S // P, KO, HD], FP16, tag="y_all")

            exp_tiles = {}

            def scores_unit(p, j, tcps=None):
                """Scores + exp for head pair p, q block j (2 heads packed
                on PE row groups; exp covers a tc pair = 1024 wide to
                amortize ScalarE per-instruction overhead)."""
                if (p, j) not in exp_tiles:
                    exp_tiles[(p, j)] = (
                        pa.tile([P, TC, BQ], FP16, tag="exps", bufs=4,
                                name=f"ea{p}_{j}"),
                        pa.tile([P, TC, BQ], FP16, tag="exps", bufs=4,
                                name=f"eb{p}_{j}"))
                ea, eb = exp_tiles[(p, j)]
                qs = slice(j * BQ, (j + 1) * BQ)
                for tcp in (range(TC // 2) if tcps is None else tcps):
                    sa = ps_sc.tile([P, 2, BQ], F32, tag="sc", name=f"sa{p}_{j}_{tcp}")
                    sb = ps_sc.tile([P, 2, BQ], F32, tag="sc", name=f"sb{p}_{j}_{tcp}")
                    for i in range(2):
                        t0 = (2 * tcp + i) * P
                        mm(sa[:, i, :], kt[0:HD, p, t0:t0 + P], qt[0:HD, p, qs])
                        mm(sb[:, i, :], kt[HD:P, p, t0:t0 + P], qt[HD:P, p, qs])
                    nc.scalar.activation(ea[:, 2 * tcp:2 * tcp + 2, :], sa[:],
                                         EXP, scale=0.125)
                    nc.scalar.activation(eb[:, 2 * tcp:2 * tcp + 2, :], sb[:],
                                         EXP, scale=0.125)

            def av_unit(p, j):
                """Flipped AV for heads 2p, 2p+1 of q block j: for each
                s-chunk accumulate y[s,0:64]+den over 16 t-chunks, then
                normalize into y_all.  Also transpose this pair's feature
                chunk of y (c == p) as soon as it is normalized."""
                ea, eb = exp_tiles.pop((p, j))
                for hi, et in ((0, ea), (1, eb)):
                    h = 2 * p + hi
                    # [P, SC, 128] fp32 pads the slot to one full bank so
                    # pool packing can't straddle a bank boundary.
                    ya = ps_av.tile([P, SC, P], F32, tag="av",
                                    name=f"ya{p}_{j}_{hi}")
                    rr = pa.tile([P, SC, 1], F32, tag="rr", bufs=2,
                                 name=f"rr{p}_{j}_{hi}")
                    for sc in range(SC):
                        for t in range(TC):
                            mm(ya[:, sc, 0:65],
                               et[:, t, sc * P:(sc + 1) * P],
                               v_blk[:, t, h, :],
                               start=(t == 0), stop=(t == TC - 1))
                        nc.vector.reciprocal(rr[:, sc, :], ya[:, sc, 64:65])
                        nc.vector.tensor_scalar_mul(
                            y_all[:, SC * j + sc, h, :], ya[:, sc, 0:HD],
                            rr[:, sc, 0:1])

            def transpose_pair(p, j, yt):
                """y_all[s, c=p features] -> yt[f, s] via PE transpose."""
                for sc in range(SC):
                    tpf = ps_pj.tile([P, BQ], F32, tag="pj",
                                     name=f"tp{p}_{j}_{sc}")
                    tph = tpf.bitcast(FP16)  # [P, 1024] fp16 view
                    nc.tensor.transpose(tph[:, 0:P],
                                        y_all[:, SC * j + sc, 2 * p:2 * p + 2, :],
                                        eye[:])
                    nc.vector.tensor_copy(yt[:, p, sc * P:(sc + 1) * P],
                                          tph[:, 0:P])

            def outproj_unit(j, yt):
                for m in range(E // P):
                    ot = ps_pj.tile([P, BQ], F32, tag="pj", name=f"ot{m}_{j}")
                    for c in range(MC):
                        mm(ot[:], wo_sb[:, c, m * P:(m + 1) * P], yt[:, c, :],
                           start=(c == 0), stop=(c == MC - 1))
                    os_ = pa.tile([P, BQ], F32, tag="ostage", bufs=2,
                                  name=f"os{m}_{j}")
                    nc.vector.tensor_copy(os_[:], ot[:])
                    nc.sync.dma_start(
                        outT[m * P:(m + 1) * P, j * BQ:(j + 1) * BQ], os_[:])

            units = [(p, j) for j in range(JB) for p in range(MC)]

            # ---------------- projections (scratch pool nested inside pa
            # so the first scores units can interleave) --------------------
            with tc.tile_pool(name="px", bufs=1) as px:
                # DMA issue order == deadline order: wq+xq0 (Q proj opens
                # the PE program), wk+xk (K proj paces the first scores),
                # wv+xv (V proj must beat the first AV unit), wo, xq rest.
                wq_sb = px.tile([P, KO, EL], FP16, tag="wq_sb")
                nc.sync.dma_start(wq_sb[:], wq3)

                def load_x(x3, nb, name):
                    xt = px.tile([P, KO, 512], FP16, tag="xt", bufs=2,
                                 name=name)
                    for k0, k1 in ((0, 1), (1, KO)):
                        nc.sync.dma_start(
                            xt[:, k0:k1, :],
                            x3[:, k0:k1, 512 * nb:512 * (nb + 1)])
                    return xt

                xq0 = load_x(xq3, 0, "xq0")
                wk_sb = px.tile([P, KO, EL], FP16, tag="wk_sb")
                nc.sync.dma_start(wk_sb[:, 0:2, :], wk3[:, 0:2, :])
                nc.sync.dma_start(wk_sb[:, 2:KO, :], wk3[:, 2:KO, :])
                nc.sync.dma_start(bq_sb[:], bq.rearrange("(m p) -> p m", p=P))
                nc.sync.dma_start(bk_sb[:], bk.rearrange("(m p) -> p m", p=P))

                def proj_mm(w_sb, out_sb, bias_sb, xt, nb, ms):
                    for m in ms:
                        pt = ps_pj.tile([P, 512], F32, tag="pj")
                        for ko in range(KO):
                            mm(pt[:], w_sb[:, ko, m * P:(m + 1) * P],
                               xt[:, ko, :], start=(ko == 0),
                               stop=(ko == KO - 1))
                        nc.vector.tensor_scalar_add(
                            out_sb[:, m, 512 * nb:512 * (nb + 1)], pt[:],
                            bias_sb[:, m:m + 1])

                proj_mm(wq_sb, qt, bq_sb, xq0, 0, range(MC))
                nc.sync.dma_start(eye[:], eyed)
                nc.sync.dma_start(bv_bc[:], bv.unsqueeze(0).to_broadcast((P, EL)))
                # K projection interleaved with the first two scores units
                # at nb granularity so exp starts ~20us in, not ~50us.
                for nb in range(NB):
                    xk_t = load_x(xk3, nb, f"xk{nb}")
                    proj_mm(wk_sb, kt, bk_sb, xk_t, nb, range(MC))
                    scores_unit(0, 0, [2 * nb, 2 * nb + 1])
                    scores_unit(1, 0, [2 * nb, 2 * nb + 1])

                # V projection into v_blk (strided per-head dest with the
                # ones column untouched)
                wv_sb = px.tile([P, KO, EL], FP16, tag="wv_sb")
                nc.sync.dma_start(wv_sb[:], wv3)
                for tb in range(4):
                    xv_t = load_x(xv3, tb, f"xv{tb}")
                    for tt in range(4):
                        t = 4 * tb + tt
                        pt = ps_pj.tile([P, EL], F32, tag="pj", name=f"vp{t}")
                        for ko in range(KO):
                            mm(pt[:], xv_t[:, ko, tt * P:(tt + 1) * P],
                               wv_sb[:, ko, :],
                               start=(ko == 0), stop=(ko == KO - 1))
                        nc.vector.tensor_add(
                            v_blk[:, t, :, 0:HD],
                            pt[:].rearrange("p (h d) -> p h d", h=KO),
                            bv_bc[:].rearrange("p (h d) -> p h d", h=KO))

                wo_sb = pa.tile([P, MC, E], FP16, tag="wo_sb")
                nc.sync.dma_start(wo_sb[:], wo3)

                # main attention loop: scores stay 2 units ahead of AV so
                # ScalarE (exp) always has queued work; Q proj for the next
                # q block is split across p==0/p==1 to avoid head-of-line
                # delays on the lookahead scores.
                yt = None
                for idx, (p, j) in enumerate(units):
                    if p == 0:
                        yt = pa.tile([P, MC, BQ], FP16, tag="yt", bufs=2,
                                     name=f"yt{j}")
                    if p == 0 and j + 1 < JB:
                        xq_hold = load_x(xq3, j + 1, f"xq{j + 1}")
                        proj_mm(wq_sb, qt, bq_sb, xq_hold, j + 1, (0, 1))
                    elif p == 1 and j + 1 < JB:
                        proj_mm(wq_sb, qt, bq_sb, xq_hold, j + 1, (2, 3))
                    av_unit(p, j)
                    transpose_pair(p, j, yt)
                    if p == MC - 1:
                        outproj_unit(j, yt)
                    if idx + 2 < len(units):
                        scores_unit(*units[idx + 2])


_NC_CACHE = None


def _build():
    global _NC_CACHE
    if _NC_CACHE is None:
        nc = bacc.Bacc("TRN2", target_bir_lowering=False, debug=False,
                       enable_asserts=False)
        with tile.TileContext(nc) as t:
            _emit(t)
        nc.compile()
        _NC_CACHE = nc
    return _NC_CACHE


def make_in_maps(query, key, value, Wq, bq, Wk, bk, Wv, bv, Wo):
    def f16(a):
        return np.ascontiguousarray(np.asarray(a, dtype=np.float32)
                                    .astype(np.float16))

    def f32(a):
        return np.ascontiguousarray(np.asarray(a, dtype=np.float32))

    bq, bk, bv = f32(bq), f32(bk), f32(bv)
    query = np.asarray(query, dtype=np.float32)
    key = np.asarray(key, dtype=np.float32)
    value = np.asarray(value, dtype=np.float32)
    Wq, Wk, Wv, Wo = (np.asarray(a, dtype=np.float32) for a in (Wq, Wk, Wv, Wo))
    eye = np.eye(P, dtype=np.float16)

    in_maps = []
    for c in range(NCORES):
        n, g = divmod(c, 2)
        cs = slice(g * EL, (g + 1) * EL)
        in_maps.append({
            "xqT": f16(query[n].T),
            "xkT": f16(key[n].T),
            "xvT": f16(value[n].T),
            "wq": f16(Wq[:, cs]),
            "wk": f16(Wk[:, cs]),
            "wv": f16(Wv[:, cs]),
            "wo": f16(Wo[cs, :]),
            "bq": np.ascontiguousarray(bq[cs]),
            "bk": np.ascontiguousarray(bk[cs]),
            "bv": np.ascontiguousarray(bv[cs]),
            "eye": eye,
        })
    return in_maps


def gather_output(results, bo):
    bo = np.asarray(bo, dtype=np.float32)
    out = np.empty((NCORES // 2, S, E), dtype=np.float32)
    for n in range(NCORES // 2):
        acc = results[2 * n]["outT"] + results[2 * n + 1]["outT"]
        out[n] = acc.T + bo
    return out


def kernel(query, key, value, Wq, bq, Wk, bk, Wv, bv, Wo, bo):
    nc = _build()
    in_maps = make_in_maps(query, key, value, Wq, bq, Wk, bk, Wv, bv, Wo)
    res = run_bass_kernel_spmd(nc, in_maps, core_ids=list(range(NCORES)))
    return gather_output(res.results, bo)
